# revision 4
# baseline (speedup 1.0000x reference)
"""AdaXbmTripletLoss kernel for 8 Trainium2 NeuronCores (Bass/Tile).

Reference math: loss = sum(hard * relu(d_ap + sqrt(margin) - d_an)) / count(hard)
with hard = ~is_nonneg & (sim > pos_sim - margin) & has_q, over [B=256, M=32768].

Device strategy (inputs_row sharded 8 ways -> ML=4096 rows/core; queries
replicated; all [B, M] work on-device in m-tiles of GM=2048):

z-space trick: host scales each query by 1/delta_b^2 (delta_b = the
d_an threshold sqrt(alpha - 2*thr)), so on device
    z = bias_b - psum/128 = d_an^2 / delta_b^2
and the mask compare becomes the GLOBAL constant 1.0:
    masked  <=>  z < 1  <=>  sqrt(z) < 1.
That removes every per-partition operand from the DVE epilogue ops, so
tensor_scalar runs with immediate scalars on bf16 data -> 4x DVE mode.

Per (g, bt) tile [128 queries x 2048 m]:
  PE:  8 matmuls fp8 DoubleRow -> psum f32 (= 256*sim/delta^2)
  ACT: sqz = Sqrt(-psum/128 + bias_b)  -> bf16 SBUF  [drains PSUM; the
       only per-element ACT pass - ACT is 1 elem/cycle/lane, ~2.4us/tile]
  DVE: count C  = accum is_lt(sqz, 1.0)      [4x mode, immediate scalar]
       Smin  S  = accum min(sqz, 1.0)        [4x mode, immediate scalar]
Host per tile: smask += delta*(S - (n - C)); total_b = gamma*count_b - smask_b.
Identity is exact per element for whatever rounding the device applied
(C and S come from the same bf16 sqz values).

DMA: three parallel issue paths (SP HWDGE, ACT HWDGE, GpSimd SWDGE) so the
row halves stream concurrently instead of FIFO-serializing on one ring.
Dummy ldweights absorb the rows-DMA waits (1-embedded-wait walrus limit);
PE clock pre-warmed with dummy matmuls during the DMA window.

Host (numpy, microseconds): index preprocessing, per-query constants in
f64, reduction of the [128, 8] per-core outputs, the sparse is_nonneg
correction (~900 of 8.4M pairs, exact f64), and exact fallbacks for
delta > gamma rows or non-finite device output (never trigger here).
"""

import os
import numpy as np
import ml_dtypes

B = 256
NCOL = 512
M = 32768
D = 512
K = 10
MARGIN = 0.1
EPS = 1e-6
TMARGIN = MARGIN ** 0.5
NCORES = 8
ML = M // NCORES          # 4096 rows per core
DCH = D // 128            # 4 contraction chunks
BT = B // 128             # 2 b-tiles
GM = 2048                 # m-tile size
G = ML // GM              # 2 groups
HM = GM // 2              # rows DMA half size

_cache = {}
last_run = {}             # exec_time_ns etc. for test harness introspection


def _patch_tile_drain():
    """This container's walrus build allows only ONE embedded sync wait per
    instruction, but TileContext's kernel-tail drain aggregates a wait per
    logical proc (engines + DMA queues) onto a single Drain instruction ->
    'Too many sync wait commands'.  Replace it with standalone single-wait
    wait_ge instructions on the sync engine followed by a bare drain."""
    import concourse.tile as tile
    from concourse.tile_sem_assignment import tick_to_sem

    if getattr(tile.TileContext, "_drain_patched", False):
        return

    def _drain_and_barrier(self, tick_clock, wait_clock):
        gc = tick_clock.global_clock
        assert self.sems is not None
        for proc_idx, sem in sorted(self.sems.allocated().items()):
            tick = gc[proc_idx]
            if tick > 0:
                self.nc.sync.wait_ge(sem, tick_to_sem(tick, proc_idx))
        self.nc.sync.drain()
        self.nc.all_engine_barrier()
        popped = self.nc._tile_sem_poison_stack.pop()
        assert popped is self._sem_poison
        self.nc.clear_and_free_semaphores(list(self.sems.allocated().values()))
        self.nc.all_engine_barrier()

    tile.TileContext._drain_and_barrier = _drain_and_barrier
    tile.TileContext._drain_patched = True


def _build_nc():
    import concourse.bass as bass
    import concourse.mybir as mybir
    import concourse.tile as tile

    _patch_tile_drain()
    nc = bass.Bass()
    f32 = mybir.dt.float32
    bf16 = mybir.dt.bfloat16
    fp8 = mybir.dt.float8e4

    # rows: per (group, half) [128, DCH, HM] fp8, 4KB contiguous per partition
    rows_ext = [
        [nc.declare_dram_parameter(f"rows{g}{h}", [128, DCH, HM], fp8, False)
         for h in range(2)]
        for g in range(G)
    ]
    q_ext = nc.declare_dram_parameter("q_t", [128, DCH, B], fp8, False)
    # consts columns: bias (= alpha/delta^2) for bt0, bt1
    consts_ext = nc.declare_dram_parameter("consts", [128, 2], f32, False)
    # out: per-(g,bt) accumulator columns [0:4] = count C, [4:8] = Smin S
    out_ext = nc.declare_dram_parameter("out", [128, 8], f32, True)

    with tile.TileContext(nc) as tc:
        with (
            tc.tile_pool(name="rows", bufs=1) as rows_pool,
            tc.tile_pool(name="qt", bufs=1) as qt_pool,
            tc.tile_pool(name="consts", bufs=1) as consts_pool,
            tc.tile_pool(name="psum", bufs=2, space="PSUM") as psum_pool,
            tc.tile_pool(name="sqz", bufs=4) as sqz_pool,
            tc.tile_pool(name="scr", bufs=2) as scr_pool,
            tc.tile_pool(name="cols", bufs=1) as cols_pool,
        ):
            rows_tiles = [
                [rows_pool.tile([128, DCH, HM], fp8, tag=f"rows{g}{h}",
                                name=f"rows{g}{h}") for h in range(2)]
                for g in range(G)
            ]
            qt_tile = qt_pool.tile([128, DCH, B], fp8)
            consts_tile = consts_pool.tile([128, 2], f32)

            # three parallel DMA issue paths; each trigger costs ~650ns of
            # sequencer time, and each HWDGE ring drains FIFO, so spread
            # the row halves across SP, ACT and the gpsimd SWDGE ring.
            nc.sync.dma_start(consts_tile[:], consts_ext[:])
            nc.sync.dma_start(qt_tile[:], q_ext[:])
            nc.scalar.dma_start(rows_tiles[0][0][:], rows_ext[0][0][:])
            nc.gpsimd.dma_start(rows_tiles[0][1][:], rows_ext[0][1][:])
            nc.sync.dma_start(rows_tiles[1][0][:], rows_ext[1][0][:])
            nc.scalar.dma_start(rows_tiles[1][1][:], rows_ext[1][1][:])

            # Warm-up: ACT sqrt on a consts column pulls the Sqrt table load
            # off the critical path and absorbs the consts-DMA wait before
            # the first real sqrt (which already carries its PE wait).
            warm = consts_pool.tile([128, 1], f32)
            nc.scalar.activation(
                warm[:], consts_tile[:, 0:1],
                mybir.ActivationFunctionType.Sqrt,
            )

            # PE clock warm-up: HAM runs the PE at 1.2GHz until ~4us of
            # sustained activity.  Dummy matmuls on scratch data while the
            # rows DMAs are in flight get the real matmuls to ~2.4GHz.
            wsrc = consts_pool.tile([128, 128], bf16)
            nc.gpsimd.memset(wsrc[:], 0.0)
            pwarm = psum_pool.tile([128, 512], f32, tag="psum", name="pwarm")
            for _ in range(7):
                nc.tensor.matmul(pwarm[:], wsrc[:], wsrc[:, 0:1].broadcast_to((128, 512)))

            cols = cols_pool.tile([128, 8], f32)

            for g in range(G):
                # dummy weight loads absorb the two rows-half-DMA waits so
                # the group's first real matmul keeps a single embedded wait
                nc.tensor.ldweights(rows_tiles[g][0][:, 0, 0:1])
                nc.tensor.ldweights(rows_tiles[g][1][:, 0, 0:1])
                for bt in range(BT):
                    t = 2 * g + bt
                    bias_ap = consts_tile[:, bt : bt + 1]
                    psum = psum_pool.tile([128, GM], f32, tag="psum",
                                          name=f"ps{g}_{bt}")
                    for dp in range(DCH // 2):
                        lhs = qt_tile[:, 2 * dp : 2 * dp + 2, bt * 128 : (bt + 1) * 128]
                        for h in range(GM // 512):
                            hsl = slice(h * 512, (h + 1) * 512)
                            rhs = rows_tiles[g][h // 2][
                                :, 2 * dp : 2 * dp + 2,
                                (h % 2) * 512 : (h % 2) * 512 + 512]
                            nc.tensor.matmul(
                                psum[:, hsl],
                                lhs,
                                rhs,
                                start=(dp == 0),
                                stop=(dp == DCH // 2 - 1),
                                perf_mode=mybir.MatmulPerfMode.DoubleRow,
                            )
                    # sqz = sqrt(bias - psum/128) = d_an/delta, in bf16 so
                    # the DVE epilogue ops hit 4x mode
                    sqz = sqz_pool.tile([128, GM], bf16, tag="sqz",
                                        name=f"sqz{g}_{bt}")
                    nc.scalar.activation(
                        sqz[:], psum[:], mybir.ActivationFunctionType.Sqrt,
                        bias=bias_ap, scale=-2.0 / 256.0,
                    )
                    # count:  C = sum 1[sqz < 1]
                    scr1 = scr_pool.tile([128, GM], bf16, tag="scr",
                                         name=f"sc{g}_{bt}")
                    nc.vector.tensor_scalar(
                        scr1[:], sqz[:], 1.0, None,
                        op0=mybir.AluOpType.is_lt,
                        op1=mybir.AluOpType.add,
                        accum_out=cols[:, t : t + 1],
                    )
                    # Smin:  S = sum min(sqz, 1) = sum_masked sqz + (n - C)
                    scr2 = scr_pool.tile([128, GM], bf16, tag="scr",
                                         name=f"sm{g}_{bt}")
                    nc.vector.tensor_scalar(
                        scr2[:], sqz[:], 1.0, None,
                        op0=mybir.AluOpType.min,
                        op1=mybir.AluOpType.add,
                        accum_out=cols[:, 4 + t : 5 + t],
                    )

            # single out DMA; all 8 accum columns are DVE-written so this
            # carries exactly one sync wait (the last DVE tick)
            nc.sync.dma_start(out_ext[:], cols[:])

    # Post-pass: the walrus build allows one embedded sync wait per
    # instruction, but Tile emits conservative same-engine self-waits (e.g.
    # a matmul's PE wait when evicting a PSUM slot, or a DVE op's DVE wait
    # when recycling a scratch buffer).  An engine executes its own queue in
    # order, so a wait on the engine's own earlier tick is always implied -
    # drop self-waits whenever another wait remains.
    _eng_prefix = {
        "PE": "PE_", "Activation": "Activation_", "DVE": "DVE_",
        "Pool": "Pool_", "SP": "SP_",
    }
    for bb in nc.m.functions[0].blocks:
        for i in bb.instructions:
            si = i.sync_info
            if si is None:
                continue
            w = si.on_wait
            if len(w) < 2:
                continue
            pref = _eng_prefix.get(getattr(i.engine, "name", None) or str(i.engine))
            if pref is None:
                continue
            keep = [x for x in w if not x.ant_name.startswith(pref)]
            if 1 <= len(keep) < len(w):
                si.on_wait = keep

    return nc


def _get_nc():
    if "nc" not in _cache:
        _cache["nc"] = _build_nc()
    return _cache["nc"]


def _install_ntff_hook():
    """The agent image's antenv lacks axon_hooks; shim it from trn_agent_boot so
    run_bass_kernel_spmd(trace=True) can capture NTFF profiles under axon."""
    import sys
    import types
    try:
        import antenv.axon_hooks  # noqa: F401
        return
    except ImportError:
        pass
    try:
        import antenv
        from trn_agent_boot.trn_boot import _ntff_profile_via_ctypes
        hook = {"h": _ntff_profile_via_ctypes("/opt/axon/libaxon_pjrt.so")}
        mod = types.ModuleType("antenv.axon_hooks")
        mod.get_axon_ntff_profile_hook = lambda: hook["h"]
        mod.set_axon_ntff_profile_hook = lambda h: hook.__setitem__("h", h)
        sys.modules["antenv.axon_hooks"] = mod
        antenv.axon_hooks = mod
    except Exception:
        pass


def kernel(inputs_col, inputs_row, targets_col, targets_row, qidxs, pidxs, nnegs, bs):
    from concourse.bass_utils import run_bass_kernel_spmd

    bs = int(np.asarray(bs))
    assert bs == B and inputs_row.shape == (M, D) and inputs_col.shape[1] == D

    inputs_col = np.asarray(inputs_col, dtype=np.float32)
    inputs_row = np.asarray(inputs_row, dtype=np.float32)
    targets_col = np.asarray(targets_col)
    targets_row = np.asarray(targets_row)
    qidxs = np.asarray(qidxs)
    nnegs = np.asarray(nnegs)

    q = inputs_col[:bs]                                        # [B, D] f32

    # ---- host-side index preprocessing (tiny int ops) ----
    match = targets_col[:bs, None] == qidxs[None, :]
    has_q = match.any(axis=1)
    qloc = match.argmax(axis=1)
    my_nnegs = nnegs[qloc]                                     # [B, K]

    pos_idx = bs + np.arange(bs)
    p = inputs_row[pos_idx]                                    # [B, D] f32

    # ---- per-query constants (f64 host math) ----
    q64 = q.astype(np.float64)
    p64 = p.astype(np.float64)
    na = (q64 * q64).sum(1)
    sa = q64.sum(1)
    # device z = (alpha - 2*sim)/delta^2 with beta_m = |r_m|^2 - 2*eps*sum(r_m)
    # ~= 1 folded in (rows are L2-normalized), so alpha includes the +1.
    alpha = na + 2.0 * EPS * sa + D * EPS * EPS + 1.0
    d_ap = np.sqrt(((q64 - p64 + EPS) ** 2).sum(1))
    gamma = d_ap + TMARGIN
    pos_sim = (q64 * p64).sum(1)
    thr = pos_sim - MARGIN
    delta2 = alpha - 2.0 * thr                 # >= 0.2 (alpha ~ 2, pos_sim <= 1)
    delta = np.where(has_q, np.sqrt(np.maximum(delta2, 1e-12)), 0.0)
    s2 = np.where(has_q, 1.0 / delta2, 0.0)
    bias = np.where(has_q, alpha * s2, 2.0)
    # rows where the masked-sum identity breaks -> exact host fallback
    bad_b = np.flatnonzero(has_q & (delta > gamma))

    # ---- device inputs ----
    # rows{g}{h} per core: [128, DCH, HM], rows[p, k, m] =
    #   inputs_row[c*ML + g*GM + h*HM + m, k*128 + p] * 16 in fp8
    rt = (inputs_row.T * np.float32(16.0)).astype(ml_dtypes.float8_e4m3)  # [D, M]
    rt = rt.reshape(DCH, 128, NCORES, G, 2, HM)             # k, p, c, g, h, m
    qp = (q64 * (16.0 * s2[:, None])).astype(np.float32)
    q_t = qp.T.astype(ml_dtypes.float8_e4m3).reshape(DCH, 128, B)
    q_t = np.ascontiguousarray(q_t.transpose(1, 0, 2))      # [128, DCH, B]
    consts = np.empty((128, 2), np.float32)
    consts[:, 0] = bias[:128]
    consts[:, 1] = bias[128:]

    in_maps = []
    for c in range(NCORES):
        rc = rt[:, :, c].transpose(2, 3, 1, 0, 4)           # [G, 2, 128, DCH, HM]
        m = {"q_t": q_t, "consts": consts}
        for g in range(G):
            for h in range(2):
                m[f"rows{g}{h}"] = np.ascontiguousarray(rc[g, h])
        in_maps.append(m)

    nc = _get_nc()
    trace = bool(os.environ.get("ATHENA_KERNEL_TRACE"))
    if trace:
        _install_ntff_hook()
    r = run_bass_kernel_spmd(nc, in_maps, list(range(NCORES)), trace=trace)
    last_run["exec_time_ns"] = r.exec_time_ns
    last_run["results"] = r

    # ---- gather partials (per-(g,bt) C and S columns) ----
    count_b = np.zeros(B, np.float64)
    smask_b = np.zeros(B, np.float64)   # sum over masked of d_an
    for c in range(NCORES):
        o = np.asarray(r.results[c]["out"], dtype=np.float64)  # [128, 8]
        for g in range(G):
            for bt in range(BT):
                t = 2 * g + bt
                sl = slice(bt * 128, (bt + 1) * 128)
                C = o[:, t]
                S = o[:, 4 + t]
                count_b[sl] += C
                # sum_masked d_an = delta * (S - (n - C))
                smask_b[sl] += delta[sl] * (S - (GM - C))
    total_b = gamma * count_b - smask_b

    # ---- exact host fallback for identity violations / non-finite output ----
    bad = set(int(b) for b in bad_b)
    nf = np.flatnonzero(~(np.isfinite(total_b) & np.isfinite(count_b)))
    bad.update(int(b) for b in nf if has_q[b])
    for b in nf:
        if not has_q[b]:
            count_b[b] = 0.0
            total_b[b] = 0.0
    if bad:
        rows64 = inputs_row.astype(np.float64)
        nb_all = (rows64 * rows64).sum(1)
        sb_all = rows64.sum(1)
        for b in sorted(bad):
            simrow = rows64 @ q64[b]
            mask = simrow > thr[b]
            d2 = (na[b] + nb_all - 2.0 * simrow
                  + 2.0 * EPS * (sa[b] - sb_all) + D * EPS * EPS)
            d_an = np.sqrt(np.maximum(d2, 0.0))
            count_b[b] = mask.sum()
            total_b[b] = np.maximum(gamma[b] - d_an, 0.0)[mask].sum()

    # ---- sparse is_nonneg correction (host, exact) ----
    order = np.argsort(targets_row, kind="stable")
    tr_sorted = targets_row[order]
    lo = np.searchsorted(tr_sorted, my_nnegs.ravel(), side="left")
    hi = np.searchsorted(tr_sorted, my_nnegs.ravel(), side="right")
    pairs = set()
    for flat, (l, h) in enumerate(zip(lo, hi)):
        if h > l:
            b = flat // K
            if has_q[b]:
                for mm_ in order[l:h]:
                    pairs.add((b, int(mm_)))
    if pairs:
        pb = np.fromiter((x[0] for x in pairs), np.int64, len(pairs))
        pm = np.fromiter((x[1] for x in pairs), np.int64, len(pairs))
        rows_sel = inputs_row[pm].astype(np.float64)
        sims = (q64[pb] * rows_sel).sum(1)
        sel = sims > thr[pb]
        pb, pm, sims, rows_sel = pb[sel], pm[sel], sims[sel], rows_sel[sel]
        nb = (rows_sel * rows_sel).sum(1)
        sb = rows_sel.sum(1)
        d2 = na[pb] + nb - 2.0 * sims + 2.0 * EPS * (sa[pb] - sb) + D * EPS * EPS
        d_an = np.sqrt(np.maximum(d2, 0.0))
        tl = np.maximum(gamma[pb] - d_an, 0.0)
        np.add.at(count_b, pb, -1.0)
        np.add.at(total_b, pb, -tl)

    neg_count = count_b.sum()
    total = total_b.sum()
    loss = total / neg_count if neg_count > 0 else 0.0
    return np.float32(loss)
